# revision 5
# baseline (speedup 1.0000x reference)
"""Trainium2 Bass kernel for nn_Memory_2774548873902 (sparse_attention).

Computes, for z:(8,2,32,32,256) fp32 and mem:(2000,256) fp32:
    z_norm = ||z||_c ; m_norm = ||mem||_c
    cos    = (z @ mem^T) / (z_norm * m_norm + 1e-14)
    w      = softmax(cos, axis=-1)
    w_hat  = relu(w - 2.5e-4)/(|w - 2.5e-4| + 1e-14) * w   (== w * [w > delta])
    w_hat  = w_hat / (sum|w_hat| + 1e-14)
    z_hat  = w_hat @ mem
Returns (z_hat, w_hat).

Strategy: data-parallel over the 16384 tokens across 8 NeuronCores (2048
tokens/core); mem (2MB) replicated.  On-chip layout keeps tokens on the
partition axis so every softmax scalar (Z, threshold, L1 sum) is a native
per-partition value:
  - mm1 (f32r): dot[t,n] = zT^T @ mhatT, accumulated over 2 k-tiles of 128.
  - ACT: e = Exp(dot * 1/z_norm_t) fused with free-dim accumulation -> Z_t.
  - DVE scalar_tensor_tensor: gated = (e > delta*Z_t) * e, fused accum -> s_t.
  - DVE tensor_scalar: w = gated * (1/s_t) -> bf16 (the w_hat output).
  - DMA xbar transposes (128x128 bf16) give w^T tiles; mm2 accumulates
    z_hatT[c,t] = sum_n mem[n,c] * w^T[n,t] over 16 n-tiles in PSUM.
mem row norms are computed on-device from memT via Square + ones-matmul
partition reduction, 1/norm via Exp(-0.5*Ln(norm^2)), and broadcast across
partitions with a K=1 matmul to normalize memT once.
"""

import numpy as np
from contextlib import ExitStack

import ml_dtypes

import concourse.bass as bass
import concourse.bacc as bacc
import concourse.tile as tile
from concourse import mybir
from concourse.bass_utils import run_bass_kernel_spmd

F32 = mybir.dt.float32
F32R = mybir.dt.float32r
BF16 = mybir.dt.bfloat16
AOP = mybir.AluOpType
AFT = mybir.ActivationFunctionType

N_CORES = 8
T_FULL = 16384          # total tokens = 8*2*32*32
T_SHARD = T_FULL // N_CORES   # 2048 tokens per core
C = 256                 # feature dim
N_MEM = 2000            # memory slots
N_PAD = 2048            # padded slots (16 clean 128-tiles for transpose/mm2)
TT = 128                # token tile (partition dim)
N_TTILES = T_SHARD // TT        # 16
N_CHUNKS = [512, 512, 512, 464]  # n chunks for mm1/psum (sum = 2000)
N_NTILES = N_PAD // 128          # 16
DELTA = 0.00025
MM2_TCHUNK = 512        # moving free dim for mm2 (bf16)

_CACHE = {}


def _build():
    nc = bacc.Bacc(None)
    zt = nc.declare_dram_parameter("zt", [C, T_SHARD], F32, isOutput=False)
    zn = nc.declare_dram_parameter("zn", [T_SHARD, C], F32, isOutput=False)
    memt = nc.declare_dram_parameter("memt", [C, N_MEM], F32, isOutput=False)
    memn = nc.declare_dram_parameter("memn", [N_PAD, C], BF16, isOutput=False)
    wout = nc.declare_dram_parameter("wout", [T_SHARD, N_MEM], BF16, isOutput=True)
    zhout = nc.declare_dram_parameter("zhout", [C, T_SHARD], F32, isOutput=True)

    with ExitStack() as ctx:
        tc = ctx.enter_context(tile.TileContext(nc))
        persist = ctx.enter_context(tc.tile_pool(name="persist", bufs=1))
        scratch = ctx.enter_context(tc.tile_pool(name="scratch", bufs=2))
        znpool = ctx.enter_context(tc.tile_pool(name="znpool", bufs=3))
        epool = ctx.enter_context(tc.tile_pool(name="epool", bufs=2))
        gpool = ctx.enter_context(tc.tile_pool(name="gpool", bufs=2))
        wpool = ctx.enter_context(tc.tile_pool(name="wpool", bufs=2))
        colpool = ctx.enter_context(tc.tile_pool(name="colpool", bufs=8))
        zhpool = ctx.enter_context(tc.tile_pool(name="zhpool", bufs=4))
        ps_dot = ctx.enter_context(tc.tile_pool(name="ps_dot", bufs=3, space="PSUM"))
        ps_misc = ctx.enter_context(tc.tile_pool(name="ps_misc", bufs=1, space="PSUM"))
        ps_zh = ctx.enter_context(tc.tile_pool(name="ps_zh", bufs=2, space="PSUM"))

        # ---------------- persistent loads ----------------
        zt_r = persist.tile([128, 2, T_SHARD], F32R, name="zt_r")
        nc.sync.dma_start(
            out=zt_r, in_=zt.rearrange("(k p) t -> p k t", p=128).bitcast(F32R)
        )
        memt_sb = persist.tile([128, 2, N_MEM], F32, name="memt_sb")
        nc.sync.dma_start(out=memt_sb, in_=memt.rearrange("(k p) n -> p k n", p=128))
        memn_sb = persist.tile([128, N_NTILES, C], BF16, name="memn_sb")
        nc.sync.dma_start(out=memn_sb, in_=memn.rearrange("(a p) c -> p a c", p=128))
        ones_col = persist.tile([128, 1], F32, name="ones_col")
        nc.vector.memset(ones_col, 1.0)
        ones_row = persist.tile([1, 128], F32, name="ones_row")
        nc.vector.memset(ones_row, 1.0)

        # ---------------- mem norms -> normalized memT (f32r) ----------------
        # norm2 row (1, N_MEM) via Square + ones-matmul partition-reduction
        inv_m_row = persist.tile([1, N_MEM], F32, name="inv_m_row")
        n_off = 0
        for ci, cw in enumerate(N_CHUNKS):
            sq = scratch.tile([128, 2, 512], F32, name="sq", tag="sq")
            n2_ps = ps_misc.tile([1, 512], F32, name="n2_ps", tag="n2")
            for k in range(2):
                nc.scalar.activation(
                    out=sq[:, k, :cw], in_=memt_sb[:, k, n_off:n_off + cw],
                    func=AFT.Square,
                )
                nc.tensor.matmul(
                    n2_ps[:, :cw], ones_col[:, :], sq[:, k, :cw],
                    start=(k == 0), stop=(k == 1),
                )
            # 1/sqrt(norm2) = Exp(-0.5 * Ln(norm2))
            ln_row = colpool.tile([1, 512], F32, name="ln_row", tag="lnr")
            nc.scalar.activation(out=ln_row[:, :cw], in_=n2_ps[:, :cw], func=AFT.Ln)
            nc.scalar.activation(
                out=inv_m_row[:, n_off:n_off + cw], in_=ln_row[:, :cw],
                func=AFT.Exp, scale=-0.5,
            )
            n_off += cw

        # mhat^T = memT * bcast(inv_m_row)  (f32r for mm1)
        mht = persist.tile([128, 2, N_MEM], F32R, name="mht")
        n_off = 0
        for ci, cw in enumerate(N_CHUNKS):
            bc_ps = ps_misc.tile([128, 512], F32, name="bc_ps", tag="bc")
            nc.tensor.matmul(
                bc_ps[:, :cw], ones_row[:, :], inv_m_row[:, n_off:n_off + cw],
                start=True, stop=True,
            )
            for k in range(2):
                nc.vector.tensor_tensor(
                    out=mht[:, k, n_off:n_off + cw],
                    in0=memt_sb[:, k, n_off:n_off + cw],
                    in1=bc_ps[:, :cw],
                    op=AOP.mult,
                )
            n_off += cw

        # w^T tiles (n-tile-major), filled by DMA transposes per token tile
        wbt = persist.tile([128, N_NTILES, T_SHARD], BF16, name="wbt")

        # ---------------- per-token-tile pipeline ----------------
        for t in range(N_TTILES):
            t0 = t * TT
            # z row norms for this token tile
            zn_t = znpool.tile([TT, C], F32, name="zn_t")
            nc.sync.dma_start(out=zn_t, in_=zn[t0:t0 + TT, :])
            zsq = znpool.tile([TT, C], F32, name="zsq")
            zn2 = colpool.tile([TT, 1], F32, name="zn2", tag="zn2")
            nc.scalar.activation(
                out=zsq, in_=zn_t, func=AFT.Square, accum_out=zn2[:, :]
            )
            znorm = colpool.tile([TT, 1], F32, name="znorm", tag="znorm")
            nc.scalar.activation(out=znorm, in_=zn2[:, :], func=AFT.Sqrt)
            inv_z = colpool.tile([TT, 1], F32, name="inv_z", tag="invz")
            nc.vector.reciprocal(inv_z[:, :], znorm[:, :])

            # mm1 (f32r) + fused exp/Z per chunk
            e_t = epool.tile([TT, N_MEM], F32, name="e_t")
            zpart = colpool.tile([TT, 4], F32, name="zpart", tag="zpart")
            chunk_offs = []
            n_off = 0
            for ci, cw in enumerate(N_CHUNKS):
                dot_ps = ps_dot.tile([TT, 512], F32, name="dot_ps", tag="dot")
                for k in range(2):
                    nc.tensor.matmul(
                        dot_ps[:, :cw],
                        zt_r[:, k, t0:t0 + TT],
                        mht[:, k, n_off:n_off + cw],
                        start=(k == 0), stop=(k == 1),
                    )
                nc.scalar.activation(
                    out=e_t[:, n_off:n_off + cw], in_=dot_ps[:, :cw], func=AFT.Exp,
                    scale=inv_z[:, :], accum_out=zpart[:, ci:ci + 1],
                )
                chunk_offs.append((n_off, cw))
                n_off += cw
            # Z = sum of 4 partials; thr = DELTA * Z
            z01 = colpool.tile([TT, 1], F32, name="z01", tag="z01")
            z23 = colpool.tile([TT, 1], F32, name="z23", tag="z23")
            nc.vector.tensor_add(z01[:, :], zpart[:, 0:1], zpart[:, 1:2])
            nc.vector.tensor_add(z23[:, :], zpart[:, 2:3], zpart[:, 3:4])
            zful = colpool.tile([TT, 1], F32, name="zful", tag="zful")
            nc.vector.tensor_add(zful[:, :], z01[:, :], z23[:, :])
            thr = colpool.tile([TT, 1], F32, name="thr", tag="thr")
            nc.scalar.mul(thr[:, :], zful[:, :], DELTA)

            # gated = (e > thr) * e, with fused L1 accumulation
            gated = gpool.tile([TT, N_MEM], BF16, name="gated")
            spart = colpool.tile([TT, 4], F32, name="spart", tag="spart")
            for ci, (off, cw) in enumerate(chunk_offs):
                nc.vector.scalar_tensor_tensor(
                    out=gated[:, off:off + cw],
                    in0=e_t[:, off:off + cw],
                    scalar=thr[:, :],
                    in1=e_t[:, off:off + cw],
                    op0=AOP.is_gt, op1=AOP.mult,
                    accum_out=spart[:, ci:ci + 1],
                )
            s01 = colpool.tile([TT, 1], F32, name="s01", tag="s01")
            s23 = colpool.tile([TT, 1], F32, name="s23", tag="s23")
            nc.vector.tensor_add(s01[:, :], spart[:, 0:1], spart[:, 1:2])
            nc.vector.tensor_add(s23[:, :], spart[:, 2:3], spart[:, 3:4])
            sful = colpool.tile([TT, 1], F32, name="sful", tag="sful")
            nc.vector.tensor_add(sful[:, :], s01[:, :], s23[:, :])
            inv_s = colpool.tile([TT, 1], F32, name="inv_s", tag="invs")
            nc.vector.reciprocal(inv_s[:, :], sful[:, :])

            # w = gated * inv_s (bf16), pad cols zeroed, DMA out + transposes
            w_bf = wpool.tile([TT, N_PAD], BF16, name="w_bf")
            nc.vector.tensor_scalar_mul(w_bf[:, :N_MEM], gated[:, :], inv_s[:, :])
            nc.vector.memset(w_bf[:, N_MEM:], 0.0)
            nc.sync.dma_start(out=wout[t0:t0 + TT, :], in_=w_bf[:, :N_MEM])
            for n in range(N_NTILES):
                nc.sync.dma_start_transpose(
                    wbt[:, n, t0:t0 + TT], w_bf[:, n * 128:(n + 1) * 128]
                )

        # ---------------- mm2: z_hatT[c,t] = sum_n mem[n,c] * w^T[n,t] -------
        for ch in range(2):  # c halves
            for tc_i in range(T_SHARD // MM2_TCHUNK):
                tc0 = tc_i * MM2_TCHUNK
                zh_ps = ps_zh.tile([128, MM2_TCHUNK], F32, name="zh_ps", tag="zh")
                for n in range(N_NTILES):
                    nc.tensor.matmul(
                        zh_ps[:, :],
                        memn_sb[:, n, ch * 128:(ch + 1) * 128],
                        wbt[:, n, tc0:tc0 + MM2_TCHUNK],
                        start=(n == 0), stop=(n == N_NTILES - 1),
                    )
                zh_sb = zhpool.tile([128, MM2_TCHUNK], F32, name="zh_sb")
                nc.scalar.copy(zh_sb, zh_ps)
                nc.sync.dma_start(
                    out=zhout[ch * 128:(ch + 1) * 128, tc0:tc0 + MM2_TCHUNK],
                    in_=zh_sb,
                )

    nc.compile()
    return nc


def _get_nc():
    if "nc" not in _CACHE:
        _CACHE["nc"] = _build()
    return _CACHE["nc"]


def prepare_inputs(z, mem):
    """Host-side shard/layout marshalling -> per-core input maps."""
    z2 = np.ascontiguousarray(z.reshape(T_FULL, C).astype(np.float32, copy=False))
    memt = np.ascontiguousarray(mem.T.astype(np.float32, copy=False))
    memn = np.zeros((N_PAD, C), dtype=ml_dtypes.bfloat16)
    memn[:N_MEM] = mem.astype(ml_dtypes.bfloat16)
    in_maps = []
    for c in range(N_CORES):
        sh = z2[c * T_SHARD:(c + 1) * T_SHARD]
        in_maps.append({
            "zt": np.ascontiguousarray(sh.T),
            "zn": sh,
            "memt": memt,
            "memn": memn,
        })
    return in_maps


def assemble_outputs(results):
    z_hat = np.empty((T_FULL, C), dtype=np.float32)
    w_hat = np.empty((T_FULL, N_MEM), dtype=np.float32)
    for c in range(N_CORES):
        r = results[c]
        z_hat[c * T_SHARD:(c + 1) * T_SHARD] = r["zhout"].T
        w_hat[c * T_SHARD:(c + 1) * T_SHARD] = r["wout"].astype(np.float32)
    z_hat = z_hat.reshape(8, 2, 32, 32, C)
    w_hat = w_hat.reshape(8, 2, 32, 32, N_MEM)
    return z_hat, w_hat


def kernel(z, mem, _trace=False, _trace_kwargs=None):
    nc = _get_nc()
    in_maps = prepare_inputs(np.asarray(z), np.asarray(mem))
    kw = dict(_trace_kwargs or {})
    if _trace and "tmpdir" not in kw:
        import tempfile
        kw["tmpdir"] = tempfile.mkdtemp(prefix="bass_trace_")
        _CACHE["trace_dir"] = kw["tmpdir"]
    out = run_bass_kernel_spmd(
        nc, in_maps, list(range(N_CORES)),
        trace=_trace, **kw,
    )
    res = assemble_outputs(out.results)
    if _trace:
        _CACHE["last_bench"] = out
    return res


# revision 13
# speedup vs baseline: 2.3472x; 2.3472x over previous
"""Trainium2 Bass kernel for nn_Memory_2774548873902 (sparse_attention).

Computes, for z:(8,2,32,32,256) fp32 and mem:(2000,256) fp32:
    cos    = (z @ mem^T) / (||z|| * ||mem|| + 1e-14)
    w      = softmax(cos, axis=-1)
    w_hat  = w * [w > delta]            (hard shrinkage, delta=2.5e-4)
    w_hat  = w_hat / sum(w_hat, -1)     (L1 renorm)
    z_hat  = w_hat @ mem
Returns (z_hat, w_hat).

Sharding: data-parallel over 16384 tokens across 8 NeuronCores (2048
tokens/core); mem replicated.  Per-core layout keeps tokens on partitions:
  - mm1 f32r: dot[t,n] = zT^T @ mhatT (mem pre-normalized on device).
  - ACT: e = Exp(dot * invznorm_t) bf16, fused free-dim accum -> Z_t.
  - DVE scalar_tensor_tensor: gated = (e > delta*Z_t)*e, fused accum -> s_t.
  - DVE tensor_scalar: w = gated * (1/s_t) -> bf16 output tile.
  - One batched DMA-xbar transpose per token tile: w(128,2048) -> wT blocks.
  - mm2 bf16: z_hatT[c,t] = sum_n mem[n,c]*wT[n,t], mem stationary,
    accumulated over 16 n-tiles in PSUM; host transposes z_hatT back.
"""

import numpy as np
from contextlib import ExitStack

import ml_dtypes

import concourse.bass as bass
import concourse.bacc as bacc
import concourse.tile as tile
from concourse import mybir
from concourse.bass_utils import run_bass_kernel_spmd

F32 = mybir.dt.float32
F32R = mybir.dt.float32r
BF16 = mybir.dt.bfloat16
AOP = mybir.AluOpType
AFT = mybir.ActivationFunctionType

N_CORES = 8
T_FULL = 16384
T_SHARD = T_FULL // N_CORES     # 2048
C = 256
N_MEM = 2000
N_PAD = 2048
TT = 128
N_TTILES = T_SHARD // TT        # 16
N_CHUNKS = [512, 512, 512, 464]
N_NTILES = N_PAD // 128         # 16
DELTA = 0.00025
MM2_TCHUNK = 512
MM2_STAGE = MM2_TCHUNK // TT    # t-tiles per mm2 stage (8)

_CACHE = {}


def _build():
    nc = bacc.Bacc(None)
    zt = nc.declare_dram_parameter("zt", [C, T_SHARD], F32, isOutput=False)
    zn = nc.declare_dram_parameter("zn", [T_SHARD, C], F32, isOutput=False)
    memt = nc.declare_dram_parameter("memt", [C, N_MEM], F32, isOutput=False)
    memn = nc.declare_dram_parameter("memn", [N_PAD, C], BF16, isOutput=False)
    wout = nc.declare_dram_parameter("wout", [T_SHARD, N_MEM], BF16, isOutput=True)
    zhout = nc.declare_dram_parameter("zhout", [C, T_SHARD], F32, isOutput=True)

    with ExitStack() as ctx:
        tc = ctx.enter_context(tile.TileContext(nc))
        persist = ctx.enter_context(tc.tile_pool(name="persist", bufs=1))
        scratch = ctx.enter_context(tc.tile_pool(name="scratch", bufs=2))
        epool = ctx.enter_context(tc.tile_pool(name="epool", bufs=2))
        gpool = ctx.enter_context(tc.tile_pool(name="gpool", bufs=2))
        wpool = ctx.enter_context(tc.tile_pool(name="wpool", bufs=3))
        znpool = ctx.enter_context(tc.tile_pool(name="znpool", bufs=4))
        colpool = ctx.enter_context(tc.tile_pool(name="colpool", bufs=4))
        zhpool = ctx.enter_context(tc.tile_pool(name="zhpool", bufs=2))
        ps_dot = ctx.enter_context(tc.tile_pool(name="ps_dot", bufs=3, space="PSUM"))
        ps_misc = ctx.enter_context(tc.tile_pool(name="ps_misc", bufs=1, space="PSUM"))
        ps_zh = ctx.enter_context(tc.tile_pool(name="ps_zh", bufs=2, space="PSUM"))

        # ---------------- persistent loads ----------------
        zt_r = persist.tile([128, 2, T_SHARD], F32R, name="zt_r")
        nc.sync.dma_start(
            out=zt_r, in_=zt.rearrange("(k p) t -> p k t", p=128).bitcast(F32R)
        )
        memt_sb = persist.tile([128, 2, N_MEM], F32, name="memt_sb")
        nc.sync.dma_start(out=memt_sb, in_=memt.rearrange("(k p) n -> p k n", p=128))
        memn_sb = persist.tile([128, N_NTILES, C], BF16, name="memn_sb")
        nc.sync.dma_start(out=memn_sb, in_=memn.rearrange("(a p) c -> p a c", p=128))
        ones_col = persist.tile([128, 1], F32, name="ones_col")
        nc.vector.memset(ones_col, 1.0)
        ones_row = persist.tile([1, 128], F32, name="ones_row")
        nc.vector.memset(ones_row, 1.0)

        # ---------------- z row norms (batched) ----------------
        zn2 = persist.tile([128, N_TTILES], F32, name="zn2")
        for t in range(N_TTILES):
            zn_t = znpool.tile([128, C], F32, name="zn_t", tag="zn_t")
            nc.sync.dma_start(out=zn_t, in_=zn[t * TT:(t + 1) * TT, :])
            zsq_t = znpool.tile([128, C], F32, name="zsq_t", tag="zsq_t")
            nc.scalar.activation(
                out=zsq_t, in_=zn_t, func=AFT.Square,
                accum_out=zn2[:, t:t + 1],
            )
        znorm = persist.tile([128, N_TTILES], F32, name="znorm")
        nc.scalar.activation(out=znorm, in_=zn2[:, :], func=AFT.Sqrt)
        inv_z = persist.tile([128, N_TTILES], F32, name="inv_z")
        nc.vector.reciprocal(inv_z[:, :], znorm[:, :])

        # ---------------- mem norms -> normalized memT (f32r) ----------------
        inv_m_row = persist.tile([1, N_MEM], F32, name="inv_m_row")
        n_off = 0
        for ci, cw in enumerate(N_CHUNKS):
            sq = scratch.tile([128, 2, 512], F32, name="sq", tag="sq")
            n2_ps = ps_misc.tile([1, 512], F32, name="n2_ps", tag="n2")
            for k in range(2):
                nc.scalar.activation(
                    out=sq[:, k, :cw], in_=memt_sb[:, k, n_off:n_off + cw],
                    func=AFT.Square,
                )
                nc.tensor.matmul(
                    n2_ps[:, :cw], ones_col[:, :], sq[:, k, :cw],
                    start=(k == 0), stop=(k == 1),
                )
            ln_row = scratch.tile([1, 512], F32, name="ln_row", tag="lnr")
            nc.scalar.activation(out=ln_row[:, :cw], in_=n2_ps[:, :cw], func=AFT.Ln)
            nc.scalar.activation(
                out=inv_m_row[:, n_off:n_off + cw], in_=ln_row[:, :cw],
                func=AFT.Exp, scale=-0.5,
            )
            n_off += cw

        mht = persist.tile([128, 2, N_MEM], F32R, name="mht")
        n_off = 0
        for ci, cw in enumerate(N_CHUNKS):
            bc_ps = ps_misc.tile([128, 512], F32, name="bc_ps", tag="bc")
            nc.tensor.matmul(
                bc_ps[:, :cw], ones_row[:, :], inv_m_row[:, n_off:n_off + cw],
                start=True, stop=True,
            )
            for k in range(2):
                nc.vector.tensor_tensor(
                    out=mht[:, k, n_off:n_off + cw],
                    in0=memt_sb[:, k, n_off:n_off + cw],
                    in1=bc_ps[:, :cw],
                    op=AOP.mult,
                )
            n_off += cw

        # w^T blocks (n-tile-major), filled by one batched transpose per t-tile
        wbt = persist.tile([128, N_NTILES, T_SHARD], BF16, name="wbt")

        # ---------------- per-token-tile pipeline ----------------
        for t in range(N_TTILES):
            t0 = t * TT
            e_t = epool.tile([TT, N_MEM], BF16, name="e_t")
            zpart = colpool.tile([TT, 4], F32, name="zpart", tag="zpart")
            chunk_offs = []
            n_off = 0
            for ci, cw in enumerate(N_CHUNKS):
                dot_ps = ps_dot.tile([TT, 512], F32, name="dot_ps", tag="dot")
                for k in range(2):
                    nc.tensor.matmul(
                        dot_ps[:, :cw],
                        zt_r[:, k, t0:t0 + TT],
                        mht[:, k, n_off:n_off + cw],
                        start=(k == 0), stop=(k == 1),
                    )
                nc.scalar.activation(
                    out=e_t[:, n_off:n_off + cw], in_=dot_ps[:, :cw], func=AFT.Exp,
                    scale=inv_z[:, t:t + 1], accum_out=zpart[:, ci:ci + 1],
                )
                chunk_offs.append((n_off, cw))
                n_off += cw
            z01 = colpool.tile([TT, 1], F32, name="z01", tag="z01")
            z23 = colpool.tile([TT, 1], F32, name="z23", tag="z23")
            nc.vector.tensor_add(z01[:, :], zpart[:, 0:1], zpart[:, 1:2])
            nc.vector.tensor_add(z23[:, :], zpart[:, 2:3], zpart[:, 3:4])
            # thr2 = DELTA * (z01 + z23) in one fused op: (z01 mult DELTA) ...
            # scalar_tensor_tensor computes (in0 op0 scalar) op1 in1, so use
            # two steps: zful = z01 + z23, thr2 = zful * DELTA
            zful = colpool.tile([TT, 1], F32, name="zful", tag="zful")
            nc.vector.tensor_add(zful[:, :], z01[:, :], z23[:, :])
            thr2 = colpool.tile([TT, 1], F32, name="thr2", tag="thr2")
            nc.vector.tensor_scalar_mul(thr2[:, :], zful[:, :], DELTA)

            gated = gpool.tile([TT, N_MEM], BF16, name="gated")
            spart = colpool.tile([TT, 4], F32, name="spart", tag="spart")
            for ci, (off, cw) in enumerate(chunk_offs):
                nc.vector.scalar_tensor_tensor(
                    out=gated[:, off:off + cw],
                    in0=e_t[:, off:off + cw],
                    scalar=thr2[:, :],
                    in1=e_t[:, off:off + cw],
                    op0=AOP.is_gt, op1=AOP.mult,
                    accum_out=spart[:, ci:ci + 1],
                )
            s01 = colpool.tile([TT, 1], F32, name="s01", tag="s01")
            s23 = colpool.tile([TT, 1], F32, name="s23", tag="s23")
            nc.vector.tensor_add(s01[:, :], spart[:, 0:1], spart[:, 1:2])
            nc.vector.tensor_add(s23[:, :], spart[:, 2:3], spart[:, 3:4])
            sful = colpool.tile([TT, 1], F32, name="sful", tag="sful")
            nc.vector.tensor_add(sful[:, :], s01[:, :], s23[:, :])
            inv_s = colpool.tile([TT, 1], F32, name="inv_s", tag="invs")
            nc.vector.reciprocal(inv_s[:, :], sful[:, :])

            w_bf = wpool.tile([TT, N_PAD], BF16, name="w_bf")
            nc.vector.tensor_scalar_mul(w_bf[:, :N_MEM], gated[:, :], inv_s[:, :])
            nc.vector.memset(w_bf[:, N_MEM:], 0.0)
            nc.sync.dma_start(out=wout[t0:t0 + TT, :], in_=w_bf[:, :N_MEM])
            # one batched xbar transpose: w (128, 2048) -> 16 (128,128) blocks
            nc.sync.dma_start_transpose(wbt[:, :, t0:t0 + TT], w_bf[:, :])

        # ---------------- mm2 staged over t halves ----------------
        for st in range(N_TTILES // MM2_STAGE):
            tc0 = st * MM2_TCHUNK
            for ch in range(2):
                zh_ps = ps_zh.tile([128, MM2_TCHUNK], F32, name="zh_ps", tag="zh")
                for n in range(N_NTILES):
                    nc.tensor.matmul(
                        zh_ps[:, :],
                        memn_sb[:, n, ch * 128:(ch + 1) * 128],
                        wbt[:, n, tc0:tc0 + MM2_TCHUNK],
                        start=(n == 0), stop=(n == N_NTILES - 1),
                    )
                zh_sb = zhpool.tile([128, MM2_TCHUNK], F32, name="zh_sb")
                nc.vector.tensor_copy(zh_sb, zh_ps)
                nc.sync.dma_start(
                    out=zhout[ch * 128:(ch + 1) * 128, tc0:tc0 + MM2_TCHUNK],
                    in_=zh_sb,
                )

    nc.compile()
    return nc


def _get_nc():
    if "nc" not in _CACHE:
        _CACHE["nc"] = _build()
    return _CACHE["nc"]


def prepare_inputs(z, mem):
    z2 = np.ascontiguousarray(z.reshape(T_FULL, C).astype(np.float32, copy=False))
    memt = np.ascontiguousarray(mem.T.astype(np.float32, copy=False))
    memn = np.zeros((N_PAD, C), dtype=ml_dtypes.bfloat16)
    memn[:N_MEM] = mem.astype(ml_dtypes.bfloat16)
    in_maps = []
    for c in range(N_CORES):
        sh = z2[c * T_SHARD:(c + 1) * T_SHARD]
        in_maps.append({
            "zt": np.ascontiguousarray(sh.T),
            "zn": sh,
            "memt": memt,
            "memn": memn,
        })
    return in_maps


def assemble_outputs(results):
    z_hat = np.empty((T_FULL, C), dtype=np.float32)
    w_hat = np.empty((T_FULL, N_MEM), dtype=np.float32)
    for c in range(N_CORES):
        r = results[c]
        z_hat[c * T_SHARD:(c + 1) * T_SHARD] = r["zhout"].T
        w_hat[c * T_SHARD:(c + 1) * T_SHARD] = r["wout"].astype(np.float32)
    return z_hat.reshape(8, 2, 32, 32, C), w_hat.reshape(8, 2, 32, 32, N_MEM)


def kernel(z, mem, _trace=False, _trace_kwargs=None):
    nc = _get_nc()
    in_maps = prepare_inputs(np.asarray(z), np.asarray(mem))
    kw = dict(_trace_kwargs or {})
    if _trace and "tmpdir" not in kw:
        import tempfile
        kw["tmpdir"] = tempfile.mkdtemp(prefix="bass_trace_")
        _CACHE["trace_dir"] = kw["tmpdir"]
    out = run_bass_kernel_spmd(
        nc, in_maps, list(range(N_CORES)),
        trace=_trace, **kw,
    )
    res = assemble_outputs(out.results)
    if _trace:
        _CACHE["last_bench"] = out
    return res
